# revision 1
# baseline (speedup 1.0000x reference)
"""Paged GQA decode attention (sparse_attention) on 8 trn2 cores.

Sharding: tensor-parallel over heads. Core c owns kv head c and q heads
4c..4c+3: column slices of Wq/Wk/Wv, row slice of Wo, head-c slice of
k_cache/v_cache. Each core computes a partial [32, 4096] o_proj output;
the host sums the 8 partials (the all-reduce of the sharding hint, done
during unshard).

Device layout choices:
  - k_cache slice is passed host-transposed as [128=hd, 65536=slots] so
    the QK^T matmul consumes gathered K chunks directly as the moving
    operand (contract dim = hd on partitions). No on-device transposes
    of K.
  - v_cache slice stays [65536, 128] (slot-major) so P@V consumes V
    chunks directly as the stationary operand (contract dim = slot).
  - scores for all 32 seqs x 4 group-heads live stacked on partitions:
    psum[4b+g, l]. Softmax runs on the full [128, 2048] tile.
  - paged gather: block_tables*BLOCK precomputed on host (int32); each
    block offset is value_load-ed into a register and used as a dynamic
    ds() DMA offset.
  - decode-token scatter: 32 column writes into kT cache + 32 row writes
    into v cache with dynamic offsets; an explicit dependency fence
    orders all gathers after all scatters.
"""

import math
import sys
from contextlib import ExitStack

import numpy as np

sys.path.insert(0, "/opt/trn_rl_repo")

B = 32
D_MODEL = 4096
H = 32
HKV = 8
HD = 128
G = H // HKV          # 4 q heads per kv head
L = 2048              # kv length per seq
BLOCK = 256
NBPS = L // BLOCK     # 8 blocks per seq
NSLOTS = 65536
EPS = 1e-6
THETA = 10000.0
SCALE = 1.0 / math.sqrt(HD)
NCORES = 8
QH = G * HD           # per-core q width = 512
USE_CRITICAL_SCATTER = True   # tile_critical scatters (fast, HW-suspect)
USE_INDIRECT_V = False         # SWDGE indirect V gather (fast, HW-suspect)

F32 = None  # filled after import
I32 = None


def build_bass(reps: int = 1):
    import concourse.bacc as bacc
    import concourse.bass as bass
    import concourse.mybir as mybir
    import concourse.tile as tile
    from concourse.masks import make_identity
    from concourse.tile import add_dep_helper

    f32 = mybir.dt.float32
    i32 = mybir.dt.int32

    nc = bacc.Bacc(None, target_bir_lowering=False)

    # ---- kernel I/O ----
    seqs_h = nc.dram_tensor("seqs_t", [D_MODEL, B], f32, kind="ExternalInput")
    wq_h = nc.dram_tensor("wq", [D_MODEL, QH], f32, kind="ExternalInput")
    wk_h = nc.dram_tensor("wk", [D_MODEL, HD], f32, kind="ExternalInput")
    wv_h = nc.dram_tensor("wv", [D_MODEL, HD], f32, kind="ExternalInput")
    wo_h = nc.dram_tensor("wo", [QH, D_MODEL], f32, kind="ExternalInput")
    qn_h = nc.dram_tensor("qn_rep", [B, QH], f32, kind="ExternalInput")
    kn_h = nc.dram_tensor("kn_rep", [B, HD], f32, kind="ExternalInput")
    cos_h = nc.dram_tensor("cos_t", [B, HD // 2], f32, kind="ExternalInput")
    sin_h = nc.dram_tensor("sin_t", [B, HD // 2], f32, kind="ExternalInput")
    kt_h = nc.dram_tensor("kt_cache", [HD, NSLOTS], f32, kind="ExternalInput")
    v_h = nc.dram_tensor("v_cache", [NSLOTS, HD], f32, kind="ExternalInput")
    bt_h = nc.dram_tensor("bt_off", [1, B * NBPS], i32, kind="ExternalInput")
    slot_h = nc.dram_tensor("slot_map", [1, B], i32, kind="ExternalInput")
    ctx_h = nc.dram_tensor("ctx_rep", [B * G, 1], i32, kind="ExternalInput")
    out_h = nc.dram_tensor("out", [B, D_MODEL], f32, kind="ExternalOutput")

    HALF = HD // 2

    with tile.TileContext(nc) as tc:
      for _rep in range(reps):
       with ExitStack() as ctx:
        cpool = ctx.enter_context(tc.tile_pool(name="const", bufs=1))
        wqp = ctx.enter_context(tc.tile_pool(name="wq", bufs=2))
        wkvp = ctx.enter_context(tc.tile_pool(name="wkv", bufs=2))
        wop = ctx.enter_context(tc.tile_pool(name="wo", bufs=3))
        ktp = ctx.enter_context(tc.tile_pool(name="kt", bufs=8))
        vp = ctx.enter_context(tc.tile_pool(name="v", bufs=2))
        ptp = ctx.enter_context(tc.tile_pool(name="pt", bufs=2))
        ptq = ctx.enter_context(tc.tile_pool(name="ptq", bufs=32))
        tmpp = ctx.enter_context(tc.tile_pool(name="tmp", bufs=2))
        outp = ctx.enter_context(tc.tile_pool(name="outs", bufs=3))
        psA = ctx.enter_context(tc.tile_pool(name="psA", bufs=3, space="PSUM"))
        psB = ctx.enter_context(tc.tile_pool(name="psB", bufs=2, space="PSUM"))
        psC = ctx.enter_context(tc.tile_pool(name="psC", bufs=1, space="PSUM"))
        psD = ctx.enter_context(tc.tile_pool(name="psD", bufs=2, space="PSUM"))

        # ---- constants / small loads ----
        ident = cpool.tile([128, 128], f32, tag="ident")
        make_identity(nc, ident[:])

        bt_sb = cpool.tile([1, B * NBPS], i32, tag="bt")
        nc.scalar.dma_start(bt_sb[:], bt_h[:, :])
        slot_sb = cpool.tile([1, B], i32, tag="slot")
        nc.scalar.dma_start(slot_sb[:], slot_h[:, :])
        cos_sb = cpool.tile([B, HALF], f32, tag="cos")
        nc.scalar.dma_start(cos_sb[:], cos_h[:, :])
        sin_sb = cpool.tile([B, HALF], f32, tag="sin")
        nc.scalar.dma_start(sin_sb[:], sin_h[:, :])
        qnw_sb = cpool.tile([B, QH], f32, tag="qnw")
        nc.scalar.dma_start(qnw_sb[:], qn_h[:, :])
        knw_sb = cpool.tile([B, HD], f32, tag="knw")
        nc.scalar.dma_start(knw_sb[:], kn_h[:, :])

        # iota + per-(b,g) valid mask  mask[p, l] = (l < ctx[p])

        # slot indices for the indirect V gather:
        #   idx_all[p, (b,j,h)] = bt_off[b,j] + 128*h + p
        ones_row = cpool.tile([1, 128], f32, tag="ones")
        nc.vector.memset(ones_row[:], 1.0)
        bt_f = cpool.tile([1, B * NBPS], f32, tag="btf")
        nc.vector.tensor_copy(bt_f[:], bt_sb[:])
        ps_bt = psB.tile([128, B * NBPS], f32, tag="tr")
        nc.tensor.matmul(ps_bt[:], lhsT=ones_row[:], rhs=bt_f[:],
                         start=True, stop=True)
        btb_f = cpool.tile([128, B * NBPS], f32, tag="btb")
        nc.scalar.copy(btb_f[:], ps_bt[:])
        iota2 = cpool.tile([128, 2], f32, tag="iota2")
        nc.gpsimd.iota(iota2[:], [[128, 2]], base=0, channel_multiplier=1,
                       allow_small_or_imprecise_dtypes=True)
        idx_f = cpool.tile([128, B * NBPS * 2], f32, tag="idxf")
        _btb = btb_f[:]
        _io2 = iota2[:]
        nc.vector.tensor_tensor(
            out=idx_f[:].rearrange("p (bj h) -> p bj h", h=2),
            in0=bass.AP(_btb.tensor, _btb.offset, list(_btb.ap) + [[0, 2]]),
            in1=bass.AP(_io2.tensor, _io2.offset,
                        [list(_io2.ap)[0], [0, B * NBPS], list(_io2.ap)[1]]),
            op=mybir.AluOpType.add)
        idx_all = cpool.tile([128, B * NBPS * 2], i32, tag="idxall")
        nc.vector.tensor_copy(idx_all[:], idx_f[:])

        # ---- seqs^T loaded directly (host provides [D_MODEL, B]) ----
        seqsT = cpool.tile([128, D_MODEL // 128 * B], f32, tag="seqsT")  # [128, 1024]
        nc.sync.dma_start(
            seqsT[:].rearrange("p (t b) -> p t b", b=B),
            bass.AP(seqs_h, 0, [[B, 128], [128 * B, D_MODEL // 128], [1, B]]))

        # ---- k/v projections first: the cache scatter (and the whole KV
        # gather stream behind it) waits only on k/v, never on q ----
        NK = D_MODEL // 128  # 32 contraction chunks
        ps_q = psC.tile([B, QH], f32, tag="accA")
        ps_k = psD.tile([B, HD], f32, tag="accB")
        ps_v = psD.tile([B, HD], f32, tag="accB")
        for m in range(4):
            wk_t = wkvp.tile([128, 8 * HD], f32, tag="wk")
            src = bass.AP(wk_h, m * 8 * 128 * HD,
                          [[HD, 128], [128 * HD, 8], [1, HD]])
            nc.scalar.dma_start(wk_t[:].rearrange("p (t d) -> p t d", d=HD), src)
            wv_t = wkvp.tile([128, 8 * HD], f32, tag="wv")
            srcv = bass.AP(wv_h, m * 8 * 128 * HD,
                           [[HD, 128], [128 * HD, 8], [1, HD]])
            nc.scalar.dma_start(wv_t[:].rearrange("p (t d) -> p t d", d=HD), srcv)
            wk3 = wk_t[:].rearrange("p (t d) -> p t d", d=HD)
            wv3 = wv_t[:].rearrange("p (t d) -> p t d", d=HD)
            for tt in range(8):
                t = m * 8 + tt
                nc.tensor.matmul(ps_k[:], lhsT=seqsT[:, t * B:(t + 1) * B],
                                 rhs=wk3[:, tt, :],
                                 start=(t == 0), stop=(t == NK - 1))
                nc.tensor.matmul(ps_v[:], lhsT=seqsT[:, t * B:(t + 1) * B],
                                 rhs=wv3[:, tt, :],
                                 start=(t == 0), stop=(t == NK - 1))

        # ---- k rmsnorm + rope + transpose (feeds the scatter) ----
        eps_t = cpool.tile([B, 1], f32, tag="eps")
        nc.vector.memset(eps_t[:], EPS)

        sqk = tmpp.tile([B, HD], f32, tag="sqk")
        nc.scalar.square(sqk[:], ps_k[:])
        ssk = tmpp.tile([B, 1], f32, tag="ssk")
        nc.vector.tensor_reduce(out=ssk[:], in_=sqk[:], axis=mybir.AxisListType.X,
                                op=mybir.AluOpType.add)
        rk = tmpp.tile([B, 1], f32, tag="rk")
        nc.scalar.activation(rk[:], ssk[:], mybir.ActivationFunctionType.Sqrt,
                             bias=eps_t[:, 0:1], scale=1.0 / HD)
        rki = tmpp.tile([B, 1], f32, tag="rki")
        nc.vector.reciprocal(rki[:], rk[:])

        kn = cpool.tile([B, HD], f32, tag="kn")
        nc.vector.tensor_scalar_mul(kn[:], ps_k[:], rki[:, 0:1])
        nc.vector.tensor_mul(kn[:], kn[:], knw_sb[:])

        v_new = cpool.tile([B, HD], f32, tag="vnew")
        nc.vector.tensor_copy(v_new[:], ps_v[:])

        def rope(dst, src, off):
            # dst/src [B, HD] slices starting at col `off`
            x1 = src[:, off:off + HALF]
            x2 = src[:, off + HALF:off + HD]
            t1 = tmpp.tile([B, HALF], f32, tag="r1")
            t2 = tmpp.tile([B, HALF], f32, tag="r2")
            nc.vector.tensor_mul(t1[:], x1, cos_sb[:])
            nc.vector.tensor_mul(t2[:], x2, sin_sb[:])
            nc.vector.tensor_sub(dst[:, off:off + HALF], t1[:], t2[:])
            nc.vector.tensor_mul(t1[:], x2, cos_sb[:])
            nc.vector.tensor_mul(t2[:], x1, sin_sb[:])
            nc.vector.tensor_add(dst[:, off + HALF:off + HD], t1[:], t2[:])

        kr = cpool.tile([B, HD], f32, tag="kr")
        rope(kr, kn, 0)

        kTn = cpool.tile([128, B], f32, tag="kTn")
        pst = psB.tile([128, B], f32, tag="tr")
        nc.tensor.transpose(pst[:], kr[:], ident[:B, :B])
        nc.vector.tensor_copy(kTn[:], pst[:])

        # Ring of reused offset registers on the sync engine. Reuse makes
        # each reg_load depend (WAR) on the previous user DMA, which both
        # bounds register pressure and stops the scheduler racing hundreds
        # of loads ahead of their DMAs.
        off_regs = [nc.sync.alloc_register(f"offr{_rep}_{i}") for i in range(6)]
        off_cnt = [0]

        def load_off(ap, lo, hi):
            r = off_regs[off_cnt[0] % len(off_regs)]
            off_cnt[0] += 1
            nc.sync.reg_load(r, ap)
            v = nc.sync.snap(r, min_val=lo, max_val=hi)
            return v


        # ---- scatter new token into caches ----
        scatter_insts = []
        if USE_CRITICAL_SCATTER:
            scat_sem = ctx.enter_context(nc.semaphore())
            with tc.tile_critical():
                for b in range(B):
                    sv = load_off(slot_sb[0:1, b:b + 1], 0, NSLOTS - 1)
                    i1 = nc.sync.dma_start(
                        kt_h[:, bass.ds(sv, 1)],
                        kTn[:, b:b + 1]).then_inc(scat_sem, 16)
                    scatter_insts.append(i1)
                for b in range(B):
                    sv = load_off(slot_sb[0:1, b:b + 1], 0, NSLOTS - 1)
                    i2 = nc.sync.dma_start(
                        v_h[bass.ds(sv, 1), :],
                        v_new[b:b + 1, :]).then_inc(scat_sem, 16)
                    scatter_insts.append(i2)
                nc.sync.wait_ge(scat_sem, 2 * B * 16)
        else:
            for b in range(B):
                sv = load_off(slot_sb[0:1, b:b + 1], 0, NSLOTS - 1)
                i1 = nc.sync.dma_start(kt_h[:, bass.ds(sv, 1)], kTn[:, b:b + 1])
                i2 = nc.sync.dma_start(v_h[bass.ds(sv, 1), :], v_new[b:b + 1, :])
                scatter_insts.append(i1)
                scatter_insts.append(i2)

        # ---- q projection/norm/rope (overlaps the scatter chain) ----
        for m in range(8):
            wq_t = wqp.tile([128, 4 * QH], f32, tag="wq")
            src = bass.AP(wq_h, m * 4 * 128 * QH,
                          [[QH, 128], [128 * QH, 4], [1, QH]])
            nc.sync.dma_start(wq_t[:].rearrange("p (t n) -> p t n", n=QH), src)
            wq3 = wq_t[:].rearrange("p (t n) -> p t n", n=QH)
            for tt in range(4):
                t = m * 4 + tt
                nc.tensor.matmul(ps_q[:], lhsT=seqsT[:, t * B:(t + 1) * B],
                                 rhs=wq3[:, tt, :],
                                 start=(t == 0), stop=(t == NK - 1))

        sqq = tmpp.tile([B, QH], f32, tag="sqq")
        nc.scalar.square(sqq[:], ps_q[:])
        ssq = tmpp.tile([B, G], f32, tag="ssq")
        nc.vector.tensor_reduce(
            out=ssq[:], in_=sqq[:].rearrange("p (g d) -> p g d", d=HD),
            axis=mybir.AxisListType.X, op=mybir.AluOpType.add)
        rq = tmpp.tile([B, G], f32, tag="rq")
        nc.scalar.activation(rq[:], ssq[:], mybir.ActivationFunctionType.Sqrt,
                             bias=eps_t[:, 0:1], scale=1.0 / HD)
        rqi = tmpp.tile([B, G], f32, tag="rqi")
        nc.vector.reciprocal(rqi[:], rq[:])
        nc.vector.tensor_scalar_mul(rqi[:], rqi[:], SCALE)

        qn = cpool.tile([B, QH], f32, tag="qn")
        for g in range(G):
            nc.vector.tensor_scalar_mul(
                qn[:, g * HD:(g + 1) * HD], ps_q[:, g * HD:(g + 1) * HD],
                rqi[:, g:g + 1])
        nc.vector.tensor_mul(qn[:], qn[:], qnw_sb[:])

        qr = cpool.tile([B, QH], f32, tag="qr")
        for g in range(G):
            rope(qr, qn, g * HD)

        qT = cpool.tile([128, 128], f32, tag="qT")
        qT3 = qT[:].rearrange("p (b g) -> p b g", g=G)
        for g in range(G):
            pst = psB.tile([128, B], f32, tag="tr")
            nc.tensor.transpose(pst[:], qr[:, g * HD:(g + 1) * HD], ident[:B, :B])
            nc.vector.tensor_copy(qT3[:, :, g], pst[:])

        if USE_CRITICAL_SCATTER:
            # ordering carried by the critical block's whole-tensor deps
            def gather_dep(inst):
                return inst
        else:
            fence = nc.sync.nop()
            for _si in scatter_insts:
                add_dep_helper(fence.ins, _si.ins, reason="fence after scatter")

            def gather_dep(inst):
                add_dep_helper(inst.ins, fence.ins, reason="gather after fence")
                return inst

        # ---- attention, pipelined over 4 groups of 8 seqs ----
        # Per group: scores^T blocks -> transpose to packed [32,(b8,g)] rows
        # -> softmax -> p^T chunks -> P@V. V gathers and next group's K
        # stream while the current group's softmax/pV run.
        GS = 8            # seqs per group
        NGRP = B // GS    # 4
        NT = 2 * NBPS     # 16 l-chunks of 128

        iota_g = cpool.tile([GS * G, L], f32, tag="iotag")
        nc.gpsimd.iota(iota_g[:], [[1, L]], base=0, channel_multiplier=0,
                       allow_small_or_imprecise_dtypes=True)
        ctx_g_tiles = []
        for grp in range(NGRP):
            cg = cpool.tile([GS * G, 1], f32, tag=f"ctxg{grp}")
            cgi = cpool.tile([GS * G, 1], i32, tag=f"ctxgi{grp}")
            nc.scalar.dma_start(cgi[:], ctx_h[grp * GS * G:(grp + 1) * GS * G, :])
            nc.vector.tensor_copy(cg[:], cgi[:])
            ctx_g_tiles.append(cg)

        ps_o = psC.tile([128, 128], f32, tag="accA")
        for grp in range(NGRP):
            # --- scores^T for this group, chunk by chunk ---
            p_grp = ptp.tile([GS * G, L], f32, tag="pgrp")
            for c in range(L // 512):
                ps_s = psA.tile([128, 128], f32, tag="scores")
                for b8 in range(GS):
                    b = grp * GS + b8
                    kt_t = ktp.tile([128, 512], f32, tag="kt")
                    for jj in range(2):
                        j = 2 * c + jj
                        ov = load_off(
                            bt_sb[0:1, b * NBPS + j:b * NBPS + j + 1],
                            0, NSLOTS - BLOCK)
                        gi = nc.sync.dma_start(
                            kt_t[:, jj * BLOCK:(jj + 1) * BLOCK],
                            kt_h[:, bass.ds(ov, BLOCK)])
                        gather_dep(gi)
                    for tt in range(4):
                        nc.tensor.matmul(
                            ps_s[:, tt * 32 + 4 * b8: tt * 32 + 4 * b8 + 4],
                            lhsT=kt_t[:, tt * 128:(tt + 1) * 128],
                            rhs=qT[:, 4 * b:4 * b + 4],
                            start=True, stop=True)
                sT_sb = tmpp.tile([128, 128], f32, tag="sT")
                nc.scalar.copy(sT_sb[:], ps_s[:])
                for tt in range(4):
                    ps_tr = psB.tile([GS * G, 128], f32, tag="tr")
                    nc.tensor.transpose(ps_tr[:], sT_sb[:, tt * 32:(tt + 1) * 32],
                                        ident[:])
                    dst = p_grp[:, c * 512 + tt * 128: c * 512 + (tt + 1) * 128]
                    if tt % 2 == 0:
                        nc.vector.tensor_copy(dst, ps_tr[:])
                    else:
                        nc.scalar.copy(dst, ps_tr[:])

            # --- softmax on packed [32, L] (no max subtraction needed:
            # rmsnormed q/k bound |scores| ~ sqrt(HD)*scale) ---
            mask_g = tmpp.tile([GS * G, L], f32, tag="maskg")
            nc.vector.tensor_scalar(
                out=mask_g[:], in0=iota_g[:], scalar1=ctx_g_tiles[grp][:, 0:1],
                scalar2=None, op0=mybir.AluOpType.is_lt)
            nc.scalar.activation(p_grp[:], p_grp[:],
                                 mybir.ActivationFunctionType.Exp)
            nc.vector.tensor_mul(p_grp[:], p_grp[:], mask_g[:])
            sm = tmpp.tile([GS * G, 1], f32, tag="sm")
            nc.vector.tensor_reduce(out=sm[:], in_=p_grp[:],
                                    axis=mybir.AxisListType.X,
                                    op=mybir.AluOpType.add)
            smr = tmpp.tile([GS * G, 1], f32, tag="smr")
            nc.vector.reciprocal(smr[:], sm[:])
            nc.vector.tensor_scalar_mul(p_grp[:], p_grp[:], smr[:, 0:1])

            # --- p^T chunks [128=l, 32=(b8,g)] ---
            ptg = []
            for t in range(NT):
                ps_t = psB.tile([128, GS * G], f32, tag="tr")
                nc.tensor.transpose(ps_t[:], p_grp[:, t * 128:(t + 1) * 128],
                                    ident[:GS * G, :GS * G])
                pt_sb = ptq.tile([128, GS * G], f32, tag="pt")
                if t % 2 == 0:
                    nc.vector.tensor_copy(pt_sb[:], ps_t[:])
                else:
                    nc.scalar.copy(pt_sb[:], ps_t[:])
                ptg.append(pt_sb)

            # --- P @ V for the group's seqs ---
            for b8 in range(GS):
                b = grp * GS + b8
                v_t = vp.tile([128, NT * HD], f32, tag="v")
                if USE_INDIRECT_V:
                    gi = nc.gpsimd.indirect_dma_start(
                        out=v_t[:].rearrange("p (t d) -> p t d", d=HD),
                        out_offset=None,
                        in_=v_h[:],
                        in_offset=bass.IndirectOffsetOnAxis(
                            ap=idx_all[:, b * NT:(b + 1) * NT], axis=0))
                    gather_dep(gi)
                else:
                    vtv = v_t[:].rearrange("p (j h d) -> p j h d", j=NBPS, h=2)
                    for j in range(NBPS):
                        ov = load_off(bt_sb[0:1, b * NBPS + j:b * NBPS + j + 1],
                                      0, NSLOTS - BLOCK)
                        srcv = v_h[bass.ds(ov, BLOCK), :].rearrange(
                            "(h l) d -> l h d", l=128)
                        gi = nc.sync.dma_start(vtv[:, j, :, :], srcv)
                        gather_dep(gi)
                v3 = v_t[:].rearrange("p (t d) -> p t d", d=HD)
                for t in range(NT):
                    nc.tensor.matmul(ps_o[:, 4 * b:4 * b + 4],
                                     lhsT=v3[:, t, :],
                                     rhs=ptg[t][:, 4 * b8:4 * b8 + 4],
                                     start=(t == 0), stop=(t == NT - 1))
        outT = cpool.tile([128, 128], f32, tag="outT")
        nc.vector.tensor_copy(outT[:], ps_o[:])
        outT3 = outT[:].rearrange("p (b g) -> p b g", g=G)

        # ---- o_proj: out[b, n] = sum_g sum_d outT[d, (b,g)] * wo[(g,d), n] ----
        for n in range(D_MODEL // 512):
            ps_out = psA.tile([B, 512], f32, tag="scores")
            wo_t = wop.tile([128, 4 * 512], f32, tag="wo")
            src = bass.AP(wo_h, n * 512,
                          [[D_MODEL, 128], [128 * D_MODEL, 4], [1, 512]])
            nc.sync.dma_start(wo_t[:].rearrange("p (g n) -> p g n", n=512), src)
            wo3 = wo_t[:].rearrange("p (g n) -> p g n", n=512)
            for g in range(G):
                nc.tensor.matmul(ps_out[:], lhsT=outT3[:, :, g],
                                 rhs=wo3[:, g, :],
                                 start=(g == 0), stop=(g == G - 1))
            o_sb = outp.tile([B, 512], f32, tag="osb")
            nc.scalar.copy(o_sb[:], ps_out[:])
            nc.sync.dma_start(out_h[:, n * 512:(n + 1) * 512], o_sb[:])

    nc.compile()
    return nc


_NC_CACHE = None


def _get_nc():
    global _NC_CACHE
    if _NC_CACHE is None:
        _NC_CACHE = build_bass()
    return _NC_CACHE


def make_in_maps(inputs):
    """Slice full inputs into 8 per-core input dicts."""
    seqs = np.asarray(inputs["seqs"], dtype=np.float32)
    Wq = np.asarray(inputs["Wq"], dtype=np.float32)
    Wk = np.asarray(inputs["Wk"], dtype=np.float32)
    Wv = np.asarray(inputs["Wv"], dtype=np.float32)
    Wo = np.asarray(inputs["Wo"], dtype=np.float32)
    qn_w = np.asarray(inputs["qn_w"], dtype=np.float32)
    kn_w = np.asarray(inputs["kn_w"], dtype=np.float32)
    k_cache = np.asarray(inputs["k_cache"], dtype=np.float32)
    v_cache = np.asarray(inputs["v_cache"], dtype=np.float32)
    input_pos = np.asarray(inputs["input_pos"], dtype=np.int32)
    slot_mapping = np.asarray(inputs["slot_mapping"], dtype=np.int32)
    block_tables = np.asarray(inputs["block_tables"], dtype=np.int32)
    context_lens = np.asarray(inputs["context_lens"], dtype=np.int32)

    half = HD // 2
    inv = (1.0 / (THETA ** (np.arange(half, dtype=np.float32) / half))).astype(
        np.float32)
    ang = input_pos.astype(np.float32)[:, None] * inv[None, :]
    cos_t = np.cos(ang).astype(np.float32)
    sin_t = np.sin(ang).astype(np.float32)

    qn_rep = np.tile(qn_w, (B, G)).astype(np.float32)        # [32, 512]
    kn_rep = np.tile(kn_w, (B, 1)).astype(np.float32)        # [32, 128]
    ctx_rep = np.repeat(context_lens, G).reshape(B * G, 1).astype(np.int32)
    bt_off = (block_tables.astype(np.int64) * BLOCK).astype(np.int32).reshape(
        1, B * NBPS)
    slot_map = slot_mapping.reshape(1, B).astype(np.int32)

    in_maps = []
    for c in range(NCORES):
        qs = slice(c * QH, (c + 1) * QH)
        ks = slice(c * HD, (c + 1) * HD)
        in_maps.append({
            "seqs_t": np.ascontiguousarray(seqs.T),
            "wq": np.ascontiguousarray(Wq[:, qs]),
            "wk": np.ascontiguousarray(Wk[:, ks]),
            "wv": np.ascontiguousarray(Wv[:, ks]),
            "wo": np.ascontiguousarray(Wo[qs, :]),
            "qn_rep": qn_rep,
            "kn_rep": kn_rep,
            "cos_t": cos_t,
            "sin_t": sin_t,
            "kt_cache": np.ascontiguousarray(k_cache[:, c, :].T),
            "v_cache": np.ascontiguousarray(v_cache[:, c, :]),
            "bt_off": bt_off,
            "slot_map": slot_map,
            "ctx_rep": ctx_rep,
        })
    return in_maps


def kernel(**inputs) -> np.ndarray:
    from concourse.bass_utils import run_bass_kernel_spmd

    nc = _get_nc()
    in_maps = make_in_maps(inputs)
    res = run_bass_kernel_spmd(nc, in_maps, core_ids=list(range(NCORES)))
    outs = [np.asarray(r["out"], dtype=np.float32) for r in res.results]
    return np.sum(np.stack(outs, axis=0), axis=0)



# revision 15
# speedup vs baseline: 4.7533x; 4.7533x over previous
"""Paged GQA decode attention (sparse_attention) on 8 trn2 cores — v2.

Sharding: tensor-parallel over heads. Core c owns kv head c and q heads
4c..4c+3: column slices of Wq/Wk/Wv, row slice of Wo, head-c slice of
k_cache/v_cache. Each core computes a partial [32, 4096] o_proj output;
the host sums the 8 partials.

v2 changes vs v1:
  - KV cache + weights cast/tiled to bf16 on the host; all attention
    matmuls run bf16 with f32 PSUM accumulation.
  - The program is specialized at trace time to the values of
    block_tables/slot_mapping/context_lens (cache keyed on them;
    rebuilds if they change). Contiguous block runs collapse to one
    512KB gather DMA per sequence per cache with 4KB descriptor rows.
  - The decode-token cache update is injected directly into the
    gathered SBUF tiles (no DRAM scatter + fence round trip).
  - V is gathered with 16 consecutive slots per partition (4KB rows);
    P^T chunks use the matching slot permutation (chunk j holds
    P[:, j::16]^T), so P@V contracts correctly with no extra moves.
  - Gather DMAs alternate between the two HWDGE rings (sync + scalar
    engines); gpsimd does constants and injections only.
"""

import math
import sys

import numpy as np
import ml_dtypes

sys.path.insert(0, "/opt/trn_rl_repo")

B = 32
D_MODEL = 4096
H = 32
HKV = 8
HD = 128
G = H // HKV          # 4 q heads per kv head
L = 2048              # kv length per seq
BLOCK = 256
NBPS = L // BLOCK     # 8 blocks per seq
NSLOTS = 65536
EPS = 1e-6
THETA = 10000.0
SCALE = 1.0 / math.sqrt(HD)
NCORES = 8
QH = G * HD           # per-core q width = 512
HALF = HD // 2
NCH = L // HD         # 16 l-chunks of 128
GS = 8                # seqs per softmax group
NGRP = B // GS        # 4


def build_bass(spec, debug=False):
    import concourse.bacc as bacc
    import concourse.bass as bass
    import concourse.mybir as mybir
    import concourse.tile as tile
    from concourse.masks import make_identity
    from contextlib import ExitStack

    seq_spec, inj_spec, ctxs = spec

    f32 = mybir.dt.float32
    bf16 = mybir.dt.bfloat16

    nc = bacc.Bacc(None, target_bir_lowering=False)

    dbg = {}
    if debug:
        dbg["qT"] = nc.dram_tensor("dbg_qT", [128, B * G], f32, kind="ExternalOutput")
        dbg["kT"] = nc.dram_tensor("dbg_kT", [128, B], f32, kind="ExternalOutput")
        dbg["vbf"] = nc.dram_tensor("dbg_vbf", [B, HD], f32, kind="ExternalOutput")
        dbg["kt0"] = nc.dram_tensor("dbg_kt0", [128, L], f32, kind="ExternalOutput")
        dbg["v0"] = nc.dram_tensor("dbg_v0", [128, L], f32, kind="ExternalOutput")
        dbg["P0"] = nc.dram_tensor("dbg_P0", [GS * G, L], f32, kind="ExternalOutput")
        dbg["pv"] = nc.dram_tensor("dbg_pv", [128, B * G], f32, kind="ExternalOutput")
        dbg["pt0"] = nc.dram_tensor("dbg_pt0", [128, GS * G], f32, kind="ExternalOutput")

    # ---- kernel I/O (all host-pre-tiled; see make_in_maps) ----
    seqs_h = nc.dram_tensor("seqs_t", [128, 32 * B], bf16, kind="ExternalInput")
    wq_h = nc.dram_tensor("wq", [128, 32 * QH], bf16, kind="ExternalInput")
    wk_h = nc.dram_tensor("wk", [128, 32 * HD], bf16, kind="ExternalInput")
    wv_h = nc.dram_tensor("wv", [128, 32 * HD], bf16, kind="ExternalInput")
    wo_h = nc.dram_tensor("wo", [128, 16384], bf16, kind="ExternalInput")
    qn_h = nc.dram_tensor("qn_rep", [B, QH], f32, kind="ExternalInput")
    kn_h = nc.dram_tensor("kn_rep", [B, HD], f32, kind="ExternalInput")
    cos_h = nc.dram_tensor("cos_t", [B, HALF], f32, kind="ExternalInput")
    sin_h = nc.dram_tensor("sin_t", [B, HALF], f32, kind="ExternalInput")
    kt_h = nc.dram_tensor("kt_cache", [HD, NSLOTS], bf16, kind="ExternalInput")
    v_h = nc.dram_tensor("v_cache", [NSLOTS, HD], bf16, kind="ExternalInput")
    out_h = nc.dram_tensor("out", [B, D_MODEL], f32, kind="ExternalOutput")

    with tile.TileContext(nc) as tc, ExitStack() as ctx:
        cpool = ctx.enter_context(tc.tile_pool(name="const", bufs=1))
        wqp = ctx.enter_context(tc.tile_pool(name="wqp", bufs=2))
        wop = ctx.enter_context(tc.tile_pool(name="wop", bufs=2))
        ktp = ctx.enter_context(tc.tile_pool(name="ktp", bufs=16))
        vp = ctx.enter_context(tc.tile_pool(name="vp", bufs=10))
        stg = ctx.enter_context(tc.tile_pool(name="stg", bufs=6))
        ptp = ctx.enter_context(tc.tile_pool(name="ptp", bufs=32))
        osb = ctx.enter_context(tc.tile_pool(name="osb", bufs=2))
        tmpp = ctx.enter_context(tc.tile_pool(name="tmp", bufs=2))
        # PSUM budget is 8 banks of [128, 2KB]; every pool tag costs
        # bufs x 1 bank here: psP 1 + psS 2 + psT 4 + psV 1 = 8.
        # ps_k/ps_v borrow psS's two banks during the projection phase —
        # interleaved accumulation groups must NOT share a bank (start=True
        # clobbers the sibling region).
        psP = ctx.enter_context(tc.tile_pool(name="psP", bufs=1, space="PSUM"))
        psS = ctx.enter_context(tc.tile_pool(name="psS", bufs=2, space="PSUM"))
        psT = ctx.enter_context(tc.tile_pool(name="psT", bufs=4, space="PSUM"))
        psV = ctx.enter_context(tc.tile_pool(name="psV", bufs=1, space="PSUM"))

        # ---- constants / small loads (gpsimd = SWDGE ring) ----
        ident = cpool.tile([128, 128], f32, tag="ident")
        make_identity(nc, ident[:])
        ident_b = cpool.tile([128, 128], bf16, tag="identb")
        nc.vector.tensor_copy(ident_b[:], ident[:])

        cos_sb = cpool.tile([B, HALF], f32, tag="cos")
        nc.gpsimd.dma_start(cos_sb[:], cos_h[:, :])
        sin_sb = cpool.tile([B, HALF], f32, tag="sin")
        nc.gpsimd.dma_start(sin_sb[:], sin_h[:, :])
        qnw_sb = cpool.tile([B, QH], f32, tag="qnw")
        nc.gpsimd.dma_start(qnw_sb[:], qn_h[:, :])
        knw_sb = cpool.tile([B, HD], f32, tag="knw")
        nc.gpsimd.dma_start(knw_sb[:], kn_h[:, :])

        eps_t = cpool.tile([B, 1], f32, tag="eps")
        nc.vector.memset(eps_t[:], EPS)

        # ---- big weight loads: wk/wv whole, seqsT ----
        seqsT = cpool.tile([128, 32 * B], bf16, tag="seqsT")
        nc.sync.dma_start(seqsT[:], seqs_h[:, :])
        wk_t = cpool.tile([128, 32 * HD], bf16, tag="wk")
        nc.sync.dma_start(wk_t[:], wk_h[:, :])
        wv_t = cpool.tile([128, 32 * HD], bf16, tag="wv")
        nc.scalar.dma_start(wv_t[:], wv_h[:, :])

        seqs3 = seqsT[:].rearrange("p (t b) -> p t b", b=B)
        wk3 = wk_t[:].rearrange("p (t d) -> p t d", d=HD)
        wv3 = wv_t[:].rearrange("p (t d) -> p t d", d=HD)

        # ---- k/v projections first (injections depend only on k/v) ----
        NK = D_MODEL // 128  # 32 contraction chunks
        ps_k = psS.tile([B, HD], f32, tag="sc")
        ps_v = psS.tile([B, HD], f32, tag="sc")
        for t in range(NK):
            nc.tensor.matmul(ps_k[:], lhsT=seqs3[:, t, :], rhs=wk3[:, t, :],
                             start=(t == 0), stop=(t == NK - 1))
            nc.tensor.matmul(ps_v[:], lhsT=seqs3[:, t, :], rhs=wv3[:, t, :],
                             start=(t == 0), stop=(t == NK - 1))

        def rope(dst, src, off):
            # dst/src [B, *] slices starting at col `off`
            x1 = src[:, off:off + HALF]
            x2 = src[:, off + HALF:off + HD]
            t1 = tmpp.tile([B, HALF], f32, tag="r1")
            t2 = tmpp.tile([B, HALF], f32, tag="r2")
            nc.vector.tensor_mul(t1[:], x1, cos_sb[:])
            nc.vector.tensor_mul(t2[:], x2, sin_sb[:])
            nc.vector.tensor_sub(dst[:, off:off + HALF], t1[:], t2[:])
            nc.vector.tensor_mul(t1[:], x2, cos_sb[:])
            nc.vector.tensor_mul(t2[:], x1, sin_sb[:])
            nc.vector.tensor_add(dst[:, off + HALF:off + HD], t1[:], t2[:])

        # k rmsnorm + rope -> kT_bf [128, 32] bf16; v -> v_bf [32, 128] bf16
        sqk = tmpp.tile([B, HD], f32, tag="sqk")
        nc.scalar.square(sqk[:], ps_k[:])
        ssk = tmpp.tile([B, 1], f32, tag="ssk")
        nc.vector.tensor_reduce(out=ssk[:], in_=sqk[:], axis=mybir.AxisListType.X,
                                op=mybir.AluOpType.add)
        rk = tmpp.tile([B, 1], f32, tag="rk")
        nc.scalar.activation(rk[:], ssk[:], mybir.ActivationFunctionType.Sqrt,
                             bias=eps_t[:, 0:1], scale=1.0 / HD)
        rki = tmpp.tile([B, 1], f32, tag="rki")
        nc.vector.reciprocal(rki[:], rk[:])

        kn = cpool.tile([B, HD], f32, tag="kn")
        nc.vector.tensor_scalar_mul(kn[:], ps_k[:], rki[:, 0:1])
        nc.vector.tensor_mul(kn[:], kn[:], knw_sb[:])
        kr = cpool.tile([B, HD], f32, tag="kr")
        rope(kr, kn, 0)

        ps_ktr = psT.tile([128, B], f32, tag="tr")
        nc.tensor.transpose(ps_ktr[:], kr[:], ident[:B, :B])
        kT_bf = cpool.tile([128, B], bf16, tag="kTbf")
        nc.vector.tensor_copy(kT_bf[:], ps_ktr[:])

        v_bf = cpool.tile([B, HD], bf16, tag="vbf")
        nc.vector.tensor_copy(v_bf[:], ps_v[:])

        # ---- q projection (wq streamed in 4 quarters) ----
        ps_q = psP.tile([B, QH], f32, tag="q")
        for m in range(4):
            wq_t = wqp.tile([128, 8 * QH], bf16, tag="wq")
            nc.scalar.dma_start(wq_t[:], wq_h[:, m * 8 * QH:(m + 1) * 8 * QH])
            wq3 = wq_t[:].rearrange("p (t n) -> p t n", n=QH)
            for tt in range(8):
                t = m * 8 + tt
                nc.tensor.matmul(ps_q[:], lhsT=seqs3[:, t, :], rhs=wq3[:, tt, :],
                                 start=(t == 0), stop=(t == NK - 1))

        sqq = tmpp.tile([B, QH], f32, tag="sqq")
        nc.scalar.square(sqq[:], ps_q[:])
        ssq = tmpp.tile([B, G], f32, tag="ssq")
        nc.vector.tensor_reduce(
            out=ssq[:], in_=sqq[:].rearrange("p (g d) -> p g d", d=HD),
            axis=mybir.AxisListType.X, op=mybir.AluOpType.add)
        rq = tmpp.tile([B, G], f32, tag="rq")
        nc.scalar.activation(rq[:], ssq[:], mybir.ActivationFunctionType.Sqrt,
                             bias=eps_t[:, 0:1], scale=1.0 / HD)
        rqi = tmpp.tile([B, G], f32, tag="rqi")
        nc.vector.reciprocal(rqi[:], rq[:])
        nc.vector.tensor_scalar_mul(rqi[:], rqi[:], SCALE)

        qn = cpool.tile([B, QH], f32, tag="qn")
        for g in range(G):
            nc.vector.tensor_scalar_mul(
                qn[:, g * HD:(g + 1) * HD], ps_q[:, g * HD:(g + 1) * HD],
                rqi[:, g:g + 1])
        nc.vector.tensor_mul(qn[:], qn[:], qnw_sb[:])
        qr = cpool.tile([B, QH], f32, tag="qr")
        for g in range(G):
            rope(qr, qn, g * HD)

        # qT_bf [128 hd, 128 (b,g)]  col 4b+g
        qT_bf = cpool.tile([128, B * G], bf16, tag="qTbf")
        qT3 = qT_bf[:].rearrange("p (b g) -> p b g", g=G)
        for g in range(G):
            ps_qtr = psT.tile([128, B], f32, tag="tr")
            nc.tensor.transpose(ps_qtr[:], qr[:, g * HD:(g + 1) * HD],
                                ident[:B, :B])
            nc.vector.tensor_copy(qT3[:, :, g], ps_qtr[:])
        if debug:
            nc.gpsimd.dma_start(dbg["qT"][:, :], qT_bf[:])
            nc.gpsimd.dma_start(dbg["kT"][:, :], kT_bf[:])
            nc.gpsimd.dma_start(dbg["vbf"][:, :], v_bf[:])

        # ---- gather issue helpers (static offsets; both HWDGE rings) ----
        def issue_k(b, eng):
            kt_t = ktp.tile([128, L], bf16, tag="kt")
            contig, offs = seq_spec[b]
            if contig:
                eng.dma_start(
                    kt_t[:],
                    bass.AP(kt_h, offs[0], [[NSLOTS, 128], [1, L]]))
            else:
                for j in range(NBPS):
                    eng.dma_start(
                        kt_t[:, j * BLOCK:(j + 1) * BLOCK],
                        bass.AP(kt_h, offs[j], [[NSLOTS, 128], [1, BLOCK]]))
            # inject new-token k columns (trace-time positions)
            for pos, i in inj_spec[b]:
                nc.gpsimd.tensor_copy(kt_t[:, pos:pos + 1], kT_bf[:, i:i + 1])
            return kt_t

        def issue_v(b, eng):
            v_t = vp.tile([128, L], bf16, tag="v")
            contig, offs = seq_spec[b]
            if contig:
                eng.dma_start(
                    v_t[:],
                    bass.AP(v_h, offs[0] * HD, [[16 * HD, 128], [1, L]]))
            else:
                for j in range(NBPS):
                    eng.dma_start(
                        v_t[j * 16:(j + 1) * 16, :],
                        bass.AP(v_h, offs[j] * HD, [[16 * HD, 16], [1, L]]))
            return v_t

        def inject_v(v_t, b):
            # single-partition compute-engine writes are illegal at
            # unaligned partitions; a tiny SBUF->SBUF DMA is not
            for pos, i in inj_spec[b]:
                p, s = pos // 16, pos % 16
                nc.gpsimd.dma_start(v_t[p:p + 1, s * HD:(s + 1) * HD],
                                    v_bf[i:i + 1, :])

        engs = [nc.sync, nc.scalar]

        pgp = ctx.enter_context(tc.tile_pool(name="pgp", bufs=2))

        ps_pv = psV.tile([128, B * G], f32, tag="pv")

        kt_cur = [issue_k(b, engs[b % 2]) for b in range(GS)]  # group 0

        for grp in range(NGRP):
            # prefetch next group's K, then this group's V
            kt_next = None
            if grp + 1 < NGRP:
                kt_next = [issue_k(grp * GS + GS + b8, engs[b8 % 2])
                           for b8 in range(GS)]
            v_cur = [issue_v(grp * GS + b8, engs[(b8 + 1) % 2])
                     for b8 in range(GS)]

            # scores^T chunks -> transpose -> exp into P rows
            P_g = pgp.tile([GS * G, L], f32, tag="pg")
            P_bfg = pgp.tile([GS * G, L], bf16, tag="pbg")
            for c in range(NCH):
                ps_c = psS.tile([128, GS * G], f32, tag="sc")
                for b8 in range(GS):
                    b = grp * GS + b8
                    nc.tensor.matmul(
                        ps_c[:, G * b8:G * b8 + G],
                        lhsT=kt_cur[b8][:, c * HD:(c + 1) * HD],
                        rhs=qT_bf[:, G * b:G * b + G],
                        start=True, stop=True)
                stg_c = stg.tile([128, GS * G], f32, tag="stg")
                if c % 2 == 0:
                    nc.vector.tensor_copy(stg_c[:], ps_c[:])
                else:
                    nc.scalar.copy(stg_c[:], ps_c[:])
                ps_tr = psT.tile([GS * G, 128], f32, tag="tr")
                nc.tensor.transpose(ps_tr[:], stg_c[:], ident[:])
                nc.scalar.activation(
                    P_g[:, c * HD:(c + 1) * HD], ps_tr[:],
                    mybir.ActivationFunctionType.Exp)

            # mask tail for any short contexts (no-op when ctx == L)
            for b8 in range(GS):
                b = grp * GS + b8
                if ctxs[b] < L:
                    nc.vector.memset(
                        P_g[G * b8:G * b8 + G, ctxs[b]:L], 0.0)

            # softmax rows for the group; scale into bf16 copy
            sm = tmpp.tile([GS * G, 1], f32, tag="sm")
            nc.vector.tensor_reduce(out=sm[:], in_=P_g[:],
                                    axis=mybir.AxisListType.X,
                                    op=mybir.AluOpType.add)
            smr = tmpp.tile([GS * G, 1], f32, tag="smr")
            nc.vector.reciprocal(smr[:], sm[:])
            nc.vector.tensor_scalar_mul(P_g[:], P_g[:], smr[:, 0:1])
            # permute cols while casting: P_bf[bg, j*128 + m] = P[bg, 16m + j]
            nc.vector.tensor_copy(
                P_bfg[:].rearrange("p (j m) -> p j m", m=128),
                P_g[:].rearrange("p (m j) -> p j m", j=16))

            # p^T chunks with the V slot permutation: pt[j][p, bg] = P[bg, 16p+j]
            pt_g = []
            for j in range(NCH):
                ps_pt = psT.tile([128, GS * G], bf16, tag="tr")
                nc.tensor.transpose(ps_pt[:], P_bfg[:, j * 128:(j + 1) * 128],
                                    ident_b[:GS * G, :GS * G])
                pt_sb = ptp.tile([128, GS * G], bf16, tag="pt")
                if j % 2 == 0:
                    nc.vector.tensor_copy(pt_sb[:], ps_pt[:])
                else:
                    nc.scalar.copy(pt_sb[:], ps_pt[:])
                pt_g.append(pt_sb)

            # P @ V
            for b8 in range(GS):
                inject_v(v_cur[b8], grp * GS + b8)
            if debug and grp == 0:
                nc.gpsimd.dma_start(dbg["pt0"][:, :], pt_g[0][:])
                nc.gpsimd.dma_start(dbg["kt0"][:, :], kt_cur[0][:])
                nc.gpsimd.dma_start(dbg["v0"][:, :], v_cur[0][:])
                nc.gpsimd.dma_start(dbg["P0"][:, :], P_g[:])
            for b8 in range(GS):
                b = grp * GS + b8
                for j in range(NCH):
                    nc.tensor.matmul(
                        ps_pv[:, G * b:G * b + G],
                        lhsT=v_cur[b8][:, j * HD:(j + 1) * HD],
                        rhs=pt_g[j][:, G * b8:G * b8 + G],
                        start=(j == 0), stop=(j == NCH - 1))

            kt_cur = kt_next

        # ---- o_proj: outT [128 d, 128 (b,g)] @ wo ----
        attn_bf = cpool.tile([128, B * G], bf16, tag="attnbf")
        nc.vector.tensor_copy(attn_bf[:], ps_pv[:])
        if debug:
            nc.gpsimd.dma_start(dbg["pv"][:, :], attn_bf[:])
        attn3 = attn_bf[:].rearrange("p (b g) -> p b g", g=G)

        for nb in range(8):
            if nb % 2 == 0:
                wo_t = wop.tile([128, 4096], bf16, tag="wo")
                engs[(nb // 2) % 2].dma_start(
                    wo_t[:], wo_h[:, nb * 2048:(nb + 2) * 2048])
                wo4 = wo_t[:].rearrange("p (h g n) -> p h g n", g=G, n=512)
            ps_o = psP.tile([B, 512], f32, tag="q")
            for g in range(G):
                nc.tensor.matmul(ps_o[:], lhsT=attn3[:, :, g],
                                 rhs=wo4[:, nb % 2, g, :],
                                 start=(g == 0), stop=(g == G - 1))
            o_sb = osb.tile([B, 512], f32, tag="osb")
            nc.scalar.copy(o_sb[:], ps_o[:])
            nc.sync.dma_start(out_h[:, nb * 512:(nb + 1) * 512], o_sb[:])

    nc.compile()
    return nc


_NC_CACHE = {}
_LAST_NC = None


def _make_spec(block_tables, slot_mapping, context_lens):
    bt_off = (block_tables.astype(np.int64) * BLOCK).astype(np.int64)
    seq_spec = []
    for b in range(B):
        offs = tuple(int(bt_off[b, j]) for j in range(NBPS))
        contig = all(offs[j] == offs[0] + j * BLOCK for j in range(NBPS))
        seq_spec.append((contig, offs))
    inj = []
    for b in range(B):
        lst = []
        for i in range(B):
            s = int(slot_mapping[i])
            for j, o in enumerate(seq_spec[b][1]):
                if o <= s < o + BLOCK:
                    lst.append((j * BLOCK + (s - o), i))
        inj.append(tuple(lst))
    ctxs = tuple(min(int(x), L) for x in context_lens)
    return (tuple(seq_spec), tuple(inj), ctxs)


def _get_nc(spec=None):
    global _LAST_NC
    if spec is None:
        assert _LAST_NC is not None
        return _LAST_NC
    if spec not in _NC_CACHE:
        _NC_CACHE[spec] = build_bass(spec)
    _LAST_NC = _NC_CACHE[spec]
    return _LAST_NC


def make_in_maps(inputs):
    """Host prep: slice per core, cast to bf16, pre-tile for flat DMAs."""
    bf = ml_dtypes.bfloat16
    seqs = np.asarray(inputs["seqs"], dtype=np.float32)
    Wq = np.asarray(inputs["Wq"], dtype=np.float32)
    Wk = np.asarray(inputs["Wk"], dtype=np.float32)
    Wv = np.asarray(inputs["Wv"], dtype=np.float32)
    Wo = np.asarray(inputs["Wo"], dtype=np.float32)
    qn_w = np.asarray(inputs["qn_w"], dtype=np.float32)
    kn_w = np.asarray(inputs["kn_w"], dtype=np.float32)
    k_cache = np.asarray(inputs["k_cache"], dtype=np.float32)
    v_cache = np.asarray(inputs["v_cache"], dtype=np.float32)
    input_pos = np.asarray(inputs["input_pos"], dtype=np.int32)

    inv = (1.0 / (THETA ** (np.arange(HALF, dtype=np.float32) / HALF))).astype(
        np.float32)
    ang = input_pos.astype(np.float32)[:, None] * inv[None, :]
    cos_t = np.cos(ang).astype(np.float32)
    sin_t = np.sin(ang).astype(np.float32)

    qn_rep = np.tile(qn_w, (B, G)).astype(np.float32)        # [32, 512]
    kn_rep = np.tile(kn_w, (B, 1)).astype(np.float32)        # [32, 128]

    # [d, b] -> [p, (t, b)] with d = t*128 + p
    seqs_tl = np.ascontiguousarray(
        seqs.T.reshape(32, 128, B).transpose(1, 0, 2).reshape(128, 32 * B)
    ).astype(bf)

    def tile_w(w, n):
        # [4096, n] -> [p, (t, n)]
        return np.ascontiguousarray(
            w.reshape(32, 128, n).transpose(1, 0, 2).reshape(128, 32 * n)
        ).astype(bf)

    in_maps = []
    for c in range(NCORES):
        qs = slice(c * QH, (c + 1) * QH)
        ks = slice(c * HD, (c + 1) * HD)
        # wo rows (g, d) -> [d, (nb, g, n)]
        wo_tl = np.ascontiguousarray(
            Wo[qs, :].reshape(G, 128, 8, 512).transpose(1, 2, 0, 3)
            .reshape(128, 16384)
        ).astype(bf)
        in_maps.append({
            "seqs_t": seqs_tl,
            "wq": tile_w(Wq[:, qs], QH),
            "wk": tile_w(Wk[:, ks], HD),
            "wv": tile_w(Wv[:, ks], HD),
            "wo": wo_tl,
            "qn_rep": qn_rep,
            "kn_rep": kn_rep,
            "cos_t": cos_t,
            "sin_t": sin_t,
            "kt_cache": np.ascontiguousarray(k_cache[:, c, :].T).astype(bf),
            "v_cache": np.ascontiguousarray(v_cache[:, c, :]).astype(bf),
        })
    return in_maps


def kernel(**inputs) -> np.ndarray:
    from concourse.bass_utils import run_bass_kernel_spmd

    spec = _make_spec(
        np.asarray(inputs["block_tables"], dtype=np.int64),
        np.asarray(inputs["slot_mapping"], dtype=np.int64),
        np.asarray(inputs["context_lens"], dtype=np.int64),
    )
    nc = _get_nc(spec)
    in_maps = make_in_maps(inputs)
    res = run_bass_kernel_spmd(nc, in_maps, core_ids=list(range(NCORES)))
    outs = [np.asarray(r["out"], dtype=np.float32) for r in res.results]
    return np.sum(np.stack(outs, axis=0), axis=0)
